# revision 48
# baseline (speedup 1.0000x reference)
"""Self-contained Trainium2 Bass kernel for the multi-head attention module.

Sharding: flat 8-way head tensor-parallelism. Core c owns heads {2c, 2c+1}
for both batches; after attention one 8-core AllToAll per head-pair index
reshards from head-space to sequence-space and each core runs the output
projection for its 512 token rows. Host concatenates the per-core row
chunks.

Precision: the QKV projections and the even-heads output projection run as
fp8e4 DoubleRow matmuls (0.5 cyc/row, 256-deep contraction) with hi+lo
residual compensation -- x@W = x_hi.W_hi + x_hi.dW + dx.W_hi, the dx.dW
term being quantization-noise-squared. x and the weights are split hi/lo
on the host at power-of-two scales that fold into the PSUM->SBUF copies,
so accuracy stays at bf16 level (~7 effective mantissa bits) while the
projections take 0.75x the bf16 PE time. Scores, exp and AV stay bf16.

Schedule: batch-major unit order so batch 1's k/v filler chunks have
units 0-7 of Act-bound slack to drain into. The AV matmul is "swapped"
(stationary = the exp tile [128 keys, 128 queries], moving = v [128, 65])
so all 128 output partitions are used: 4160 PE cycles per unit instead of
8192, with the softmax denominator landing per-query-partition -- the
normalization is a [128,4] reciprocal plus four per-partition scalar
multiplies, and a PE transpose (identity stationary, in the ps_a psum
rotation) restores the [v, query] layout the collectives consume. The
transposes run two key-blocks into the next unit so they never wait on
the DVE. The even-heads AllToAll fires after unit 11; its projection pass
is saved for the end phase where it bridges the odd-heads collective
latency and keeps the PE clock at 2.4 GHz for the final pass, whose
PSUM->SBUF copies run on the by-then-idle Act engine.
"""

import sys

sys.path.insert(0, "/opt/trn_rl_repo")

from collections import deque

import ml_dtypes
import numpy as np

from concourse import bacc, bass_utils, mybir, tile

B, S, D, H, DK, DV, DO = 2, 2048, 1024, 16, 64, 64, 1024
T = B * S          # 4096 flattened tokens
NCORES = 8
HPC = H // NCORES  # 2 heads per core
ROWS = T // NCORES # 512 output rows per core
TCH = 512          # token chunk for projections / q chunks
F32 = mybir.dt.float32
F32R = mybir.dt.float32r
BF16 = mybir.dt.bfloat16
FP8 = mybir.dt.float8e4
DR = mybir.MatmulPerfMode.DoubleRow
EXP = mybir.ActivationFunctionType.Exp
# fp8 hi+lo pre-scales: x is quantized at 4x, W at 256x, so the QKV psum
# carries q/k/v * 1024 and the PSUM->SBUF copies fold in 1/1024.
SX, SW = 4.0, 256.0
QKV_SCALE = 1.0 / (SX * SW)

_cache = {}


def _build(collective=True):
    nc = bacc.Bacc("TRN2", target_bir_lowering=False, debug=False,
                   num_devices=NCORES if collective else 1)
    # x^T as fp8 hi/lo pairs interleaved per d-block: row (dc*2+hl)*128+p
    # holds x_hi (hl=0) or the e4m3 residual (hl=1) of d-row dc*128+p --
    # same total bytes as one bf16 copy, and the interleave makes the
    # chunk DMA a single 3-dim strided copy
    xt_d = nc.dram_tensor("xt", [2 * D, T], FP8, kind="ExternalInput").ap()
    # host-pre-shuffled [128, 3*8*2*128]: per (proj n, d-block dc) the
    # two 128-col groups [W_hi | dW]; DoubleRow pairs are always formed
    # across adjacent d-blocks so no operand needs duplicating
    wqkv_d = nc.dram_tensor("wqkv", [128, 3 * 8 * 2 * 128], FP8,
                            kind="ExternalInput").ap()
    wo_d = nc.dram_tensor("wo", [H * DV, DO], BF16, kind="ExternalInput").ap()
    # pass-0 output projection weights in fp8 hi/lo: wo8[k, p*2048 +
    # hl*1024 + n] = (hi|lo of 256*Wo)[(4p + 2*(k//64))*64 + k%64, n]
    wo8_d = nc.dram_tensor("wo8", [128, 4 * 2 * DO], FP8,
                           kind="ExternalInput").ap()
    ident_d = nc.dram_tensor("ident", [128, 128], BF16,
                             kind="ExternalInput").ap()
    out_d = nc.dram_tensor("out", [ROWS, DO], BF16, kind="ExternalOutput").ap()
    bnc_in0 = nc.dram_tensor("bnc_in0", [NCORES, 64, ROWS], BF16).ap()
    bnc_out0 = nc.dram_tensor("bnc_out0", [NCORES, 64, ROWS], BF16).ap()
    # the second head-pair's resharding is split into two row-half
    # collectives so the output projection can start on the first half
    # while the second is still in flight
    bnc_in1 = [nc.dram_tensor(f"bnc_in1_{r}", [NCORES, 64, ROWS // 2],
                              BF16).ap() for r in range(2)]
    bnc_out1 = [nc.dram_tensor(f"bnc_out1_{r}", [NCORES, 64, ROWS // 2],
                               BF16).ap() for r in range(2)]

    with tile.TileContext(nc) as tc:
        with (
            tc.tile_pool(name="sb", bufs=1) as sb,
            tc.tile_pool(name="ps", bufs=1, space="PSUM") as ps,
            nc.allow_low_precision(reason="bf16 compute is intentional"),
        ):
            # constants: warm-up operand, identity for the PE transposes,
            # and a zero row for the AV-accumulator clearing matmul
            ones_f = sb.tile([128, 64], F32, tag="onesf", bufs=1)
            nc.vector.memset(ones_f[:], 1.0)
            ones_b = sb.tile([128, 64], F32R, tag="ones", bufs=1)
            nc.vector.tensor_copy(ones_b[:], ones_f[:])
            ident = sb.tile([128, 128], BF16, tag="ident", bufs=1)

            # HWDGE descriptor generation costs ~625ns per DMA instruction,
            # serialized, so inputs are fetched with as few fat strided DMAs
            # as possible. The first x chunk is interleaved with the weights
            # so phase 1 can start a few us in.
            wqkv_sb = sb.tile([128, 3 * 8 * 2 * 128], FP8, tag="wqkv",
                              bufs=1)
            _wn = {"q": 0, "k": 1, "v": 2}

            def wv_view():
                return wqkv_sb[:].rearrange(
                    "p (n dc hl c) -> p n dc hl c",
                    n=3, dc=8, hl=2)

            # x^T fp8 chunks: [128, dc(8) x (hi|lo)(2) x 512] per chunk
            xTc = [sb.tile([128, 8 * 2 * TCH], FP8, tag="xTc", bufs=8,
                           name=f"xTc{tci}") for tci in range(8)]

            def xv(tci):
                return xTc[tci][:].rearrange(
                    "p (dc hl t) -> p dc hl t", hl=2, t=TCH)

            def load_x_chunk(tci, half, width=4):
                c0 = tci * TCH
                dc0 = half * width
                nc.sync.dma_start(
                    xTc[tci][:, dc0 * 2 * TCH:(dc0 + width) * 2
                             * TCH].rearrange("p (b t) -> p b t", t=TCH),
                    xt_d[dc0 * 256:(dc0 + width) * 256,
                         c0:c0 + TCH].rearrange(
                        "(b p) t -> p b t", p=128))

            # q weights first (phase 1 starts with the q projection), the
            # first x chunk in quarters interleaved so its first d-blocks
            # land early, k weights right after the first quarter (q0 and
            # k0 emission is interleaved), then v weights and the rest of
            # x in halves
            nc.sync.dma_start(wqkv_sb[:, 0:2048], wqkv_d[:, 0:2048])
            load_x_chunk(0, 0, width=2)
            nc.sync.dma_start(wqkv_sb[:, 2048:4096], wqkv_d[:, 2048:4096])
            for quarter in range(1, 4):
                load_x_chunk(0, quarter, width=2)
            nc.sync.dma_start(wqkv_sb[:, 4096:6144], wqkv_d[:, 4096:6144])
            nc.sync.dma_start(ident[:], ident_d[:])
            for tci in range(1, 8):
                load_x_chunk(tci, 0)
                load_x_chunk(tci, 1)

            # pass-1 wo pair tiles (bf16): heads {4p+1, 4p+3} stacked on
            # partitions, matching the oTf layout; pass 0 uses the fp8
            # hi/lo wo8 tile instead (one fat DMA)
            wo_p = {1: []}
            for p in range(4):
                wt = sb.tile([128, DO], BF16, tag="wo", bufs=4,
                             name=f"wo1_{p}")
                for half, head in ((0, 4 * p + 1), (1, 4 * p + 2 + 1)):
                    nc.sync.dma_start(
                        wt[half * 64:half * 64 + 64, :],
                        wo_d[head * 64:head * 64 + 64, :])
                wo_p[1].append(wt)
            wo8 = sb.tile([128, 4 * 2 * DO], FP8, tag="wo8", bufs=1)
            nc.sync.dma_start(wo8[:], wo8_d[:])

            def wo8v():
                return wo8[:].rearrange("k (p hl n) -> k p hl n",
                                        p=4, hl=2)

            # persistent activations
            qT = sb.tile([128, T], BF16, tag="qT", bufs=1)
            kT = sb.tile([128, T], BF16, tag="kT", bufs=1)
            # v in natural [token, v] layout: 32 t-blocks x (2 heads x
            # [64 v cols | ones]) -> AV moving slices [128, 65]
            v_dual = sb.tile([128, 32 * 130], BF16, tag="vdual", bufs=1)
            ones_cols = v_dual[:].rearrange(
                "p (b h c) -> p b h c", h=2, c=65)[:, :, :, 64:65]
            nc.vector.memset(ones_cols, 1.0)

            # ---- filler queue: small PE thunks rationed into the
            # Act-bound slack of the attention inner loop ----
            fill_q = deque()  # entries: (cost_ns, label, thunk)

            def filler_slot(budget=200):
                spent = 0
                while fill_q and spent < budget:
                    cost, _, thunk = fill_q.popleft()
                    thunk()
                    spent += cost

            def flush_through(label):
                while any(e[1] == label for e in fill_q):
                    _, _, thunk = fill_q.popleft()
                    thunk()

            def flush_all():
                while fill_q:
                    fill_q.popleft()[2]()

            # ---- phase 1 parts, fp8 DoubleRow with hi+lo compensation:
            # x@W = x_hi.W_hi + x_hi.dW + dx.W_hi (the dx.dW term is
            # quantization-noise-squared and dropped). Each DoubleRow
            # matmul contracts two 128-deep groups at 0.5 cyc/row, so a
            # chunk-projection is 12 DRs instead of 8 bf16 matmuls.
            def qk_part_thunks(tci, name):
                holder = {}
                c0 = tci * TCH
                n = _wn[name]
                # per adjacent d-block pair: W_hi.x_hi, W_hi.x_lo and
                # dW.x_hi, each as one across-pair DoubleRow
                steps = [(kind, dc) for dc in range(0, 8, 2)
                         for kind in ("p0", "p3", "p4")]

                def mk(si):
                    kind, dc = steps[si]

                    def t():
                        if si == 0:
                            holder["pp"] = ps.tile(
                                [128, TCH], F32, tag="ps_a", bufs=2,
                                name=f"pp{tci}_{name}")
                        w = wv_view()
                        whl = 1 if kind == "p4" else 0
                        xhl = 1 if kind == "p3" else 0
                        lhs = w[:, n, dc:dc + 2, whl, :]
                        rhs = xv(tci)[:, dc:dc + 2, xhl, :]
                        nc.tensor.matmul(
                            holder["pp"][:], lhs, rhs, perf_mode=DR,
                            start=(si == 0), stop=(si == len(steps) - 1))
                        if si == len(steps) - 1:
                            dst = qT if name == "q" else kT
                            nc.vector.tensor_scalar_mul(
                                dst[:, c0:c0 + TCH], holder["pp"][:],
                                QKV_SCALE)
                    return t
                return [(107, f"{name}{tci}", mk(si))
                        for si in range(len(steps))]

            def v_part_thunks(tci):
                holder = {}
                steps = [(kind, dc) for dc in range(0, 8, 2)
                         for kind in ("p0", "p3", "p4")]

                def mk(tb, si):
                    kind, dc = steps[si]

                    def t():
                        if tb == 0 and si == 0:
                            holder["pv"] = ps.tile(
                                [128, TCH], F32, tag="ps_a", bufs=2,
                                name=f"pv{tci}")
                        w = wv_view()
                        whl = 1 if kind == "p3" else 0
                        xhl = 1 if kind == "p4" else 0
                        lhs = xv(tci)[:, dc:dc + 2, xhl,
                                      tb * 128:(tb + 1) * 128]
                        rhs = w[:, 2, dc:dc + 2, whl, :]
                        nc.tensor.matmul(
                            holder["pv"][:, tb * 128:(tb + 1) * 128],
                            lhs, rhs, perf_mode=DR,
                            start=(si == 0), stop=(si == len(steps) - 1))
                        if tb == 3 and si == len(steps) - 1:
                            vd = v_dual[:, tci * 4 * 130:
                                        (tci + 1) * 4 * 130].rearrange(
                                "p (b h c) -> p b h c", h=2, c=65)[
                                :, :, :, 0:64]
                            nc.vector.tensor_scalar_mul(
                                vd, holder["pv"][:].rearrange(
                                    "p (b h c) -> p b h c", h=2, c=64),
                                QKV_SCALE)
                    return t
                return [(27, f"v{tci}", mk(tb, si))
                        for tb in range(4) for si in range(len(steps))]

            # ---- attention units, batch-major (h inside batch): batch
            # 1's k/v chunks are not needed until unit 8, giving the
            # filler queue twice the runway to drain them; the even-heads
            # AllToAll then fires after unit 11, which is still early
            # enough since pass 0 runs in the end phase anyway.
            sched = [(b, h, qc) for b in range(B) for h in range(HPC)
                     for qc in range(S // TCH)]

            # deferred tail: part A (DVE recip + normalize) runs at unit
            # end; part B (PE transposes + obc copy + ship) runs two key
            # blocks into the next unit so the transposes never wait on
            # the DVE.
            pending_tail_b = [None]

            def emit_tail_a(b, h, qc, po):
                povr = po[:].rearrange("p (qb c) -> p qb c", c=65)
                bc = sb.tile([128, 4], F32, tag="bc", bufs=2,
                             name=f"bc{b}_{h}_{qc}")
                obsb = sb.tile([128, 4 * 64], BF16, tag="obsb", bufs=2,
                               name=f"obsb{b}_{h}_{qc}")
                nc.vector.reciprocal(
                    bc[:, :].rearrange("p (a c) -> p a c", c=1),
                    povr[:, :, 64:65])
                for qb in range(4):
                    nc.vector.tensor_scalar_mul(
                        obsb[:, qb * 64:(qb + 1) * 64],
                        povr[:, qb, 0:64], bc[:, qb:qb + 1])
                pending_tail_b[0] = (b, h, qc, obsb)

            def emit_tail_b():
                if pending_tail_b[0] is None:
                    return
                b, h, qc, obsb = pending_tail_b[0]
                pending_tail_b[0] = None
                shard = b * (S // TCH) + qc
                obc = sb.tile([64, TCH], BF16, tag="obc", bufs=3,
                              name=f"obc{b}_{h}_{qc}")
                for qb0, qb1 in ((0, 4),):
                    # transpose [128 q, 64 v] -> [64 v, 128 q]; pt borrows
                    # a ps_a rotation slot (never the score-exp slots)
                    pt = ps.tile([64, (qb1 - qb0) * 128], BF16, tag="ps_a",
                                 bufs=2, name=f"pt{b}_{h}_{qc}_{qb0}")
                    for qb in range(qb0, qb1):
                        nc.tensor.matmul(
                            pt[:, (qb - qb0) * 128:(qb - qb0 + 1) * 128],
                            obsb[:, qb * 64:(qb + 1) * 64], ident[:],
                            is_transpose=True, start=True, stop=True)
                    c0, c1 = qb0 * 128, qb1 * 128
                    nc.vector.tensor_copy(obc[:, c0:c1], pt[:])
                    if h == 0:
                        if qb1 == 4:
                            nc.sync.dma_start(bnc_in0[shard, :, :], obc[:])
                    else:
                        for r in range(2):
                            r0, r1 = r * (TCH // 2), (r + 1) * (TCH // 2)
                            if r0 >= c0 and r1 <= c1:
                                nc.sync.dma_start(bnc_in1[r][shard, :, :],
                                                  obc[:, r0:r1])

            # The 16 attention units run as one globally software-pipelined
            # stream: the scores+exp block always runs exactly two
            # key-blocks ahead of the AV matmuls.
            ex_store = {}
            pre_s_hook = {}

            def emit_s(u, i):
                if (u, i) in pre_s_hook:
                    pre_s_hook.pop((u, i))()
                b, h, qc = sched[u]
                qoff = b * S + qc * TCH
                pscr = ps.tile([128, 2 * TCH], F32, tag="ps_s", bufs=2,
                               name=f"pscr{u}_{i}")
                for j in range(2):
                    koff = b * S + (2 * i + j) * 128
                    nc.tensor.matmul(
                        pscr[:, j * TCH:(j + 1) * TCH],
                        kT[h * 64:(h + 1) * 64, koff:koff + 128],
                        qT[h * 64:(h + 1) * 64, qoff:qoff + TCH],
                        start=True, stop=True)
                ex = sb.tile([128, 2 * TCH], BF16, tag="ex", bufs=6,
                             name=f"ex{u}_{i}")
                nc.scalar.activation(ex[:], pscr[:], EXP, scale=0.125)
                ex_store[(u, i)] = ex

            at_hook = {}

            def run_units():
                emit_s(0, 0)
                emit_s(0, 1)
                for u in range(len(sched)):
                    b, h, qc = sched[u]
                    po = ps.tile([128, 4 * 65], F32, tag="ps_o", bufs=2,
                                 name=f"po{u}")
                    # the first AV matmul's start=True marks the whole 2KB
                    # PSUM zero-region pending, so the other query-block
                    # groups lazily zero-fill on their first write and all
                    # four accumulate correctly with start=False
                    for i in range(8):
                        for hk in at_hook.pop((u, i), ()):
                            hk()
                        ex = ex_store.pop((u, i))
                        for j in range(2):
                            kb = 2 * i + j
                            blk = b * 16 + kb
                            vmov = v_dual[:, blk * 130 + h * 65:
                                          blk * 130 + h * 65 + 65]
                            for qb in range(4):
                                nc.tensor.matmul(
                                    po[:, qb * 65:(qb + 1) * 65],
                                    ex[:, j * TCH + qb * 128:
                                       j * TCH + (qb + 1) * 128],
                                    vmov,
                                    start=(kb == 0 and qb == 0),
                                    stop=(kb == S // 128 - 1 and qb == 3),
                                    skip_group_check=True)
                        if i + 2 < 8:
                            emit_s(u, i + 2)
                        elif u + 1 < len(sched):
                            emit_s(u + 1, i - 6)
                        if i == 2:
                            # previous unit's transposes: its normalize
                            # DVE ops have had two key-blocks to finish
                            emit_tail_b()
                        filler_slot(budget=340)
                    emit_tail_a(b, h, qc, po)

            def emit_a2a0():
                if collective:
                    nc.gpsimd.collective_compute(
                        "AllToAll", mybir.AluOpType.bypass,
                        replica_groups=[list(range(NCORES))],
                        ins=[bnc_in0[:]], outs=[bnc_out0[:]])
                else:
                    nc.sync.dma_start(bnc_out0[:], bnc_in0[:])

            def emit_a2a1(r):
                if collective:
                    nc.gpsimd.collective_compute(
                        "AllToAll", mybir.AluOpType.bypass,
                        replica_groups=[list(range(NCORES))],
                        ins=[bnc_in1[r][:]], outs=[bnc_out1[r][:]])
                else:
                    nc.sync.dma_start(bnc_out1[r][:], bnc_in1[r][:])

            # ---- output projection pass h: heads {4p+h, 4p+2+h};
            # oTf[h][j*64+r, p*512+t] = bnc_out[h][2p+j, r, t]
            oTf = {}

            def emit_oTf0():
                t = sb.tile([128, 4 * ROWS], BF16, tag="oTf", bufs=3,
                            name="oTf0")
                nc.sync.dma_start(
                    t[:].rearrange("q (p tt) -> q p tt", tt=ROWS),
                    bnc_out0[:].rearrange(
                        "(p j) r tt -> (j r) p tt", j=2))
                oTf[0] = t

            def emit_oTf1(r):
                hw = ROWS // 2
                t = sb.tile([128, 4 * hw], BF16, tag="oTf", bufs=3,
                            name=f"oTf1_{r}")
                # two half-loads so pass 1's first stationary blocks are
                # ready before the second half of the transfer lands
                for ph in range(2):
                    nc.sync.dma_start(
                        t[:, 2 * ph * hw:2 * (ph + 1) * hw].rearrange(
                            "q (p tt) -> q p tt", tt=hw),
                        bnc_out1[r][:].rearrange(
                            "(p j) rr tt -> (j rr) p tt", j=2)[
                            :, 2 * ph:2 * (ph + 1), :])
                oTf[(1, r)] = t

            o0sb = [sb.tile([128, 512], BF16, tag="o0sb", bufs=8,
                            name=f"o0sb{i}") for i in range(8)]
            # fp8 hi/lo requant of the even-heads oTf, filled per p-block
            # on the idle DVE during units 13-14
            oTf8 = sb.tile([128, 4 * 2 * TCH], FP8, tag="oTf8", bufs=1)

            def oTf8v():
                return oTf8[:].rearrange("q (p hl t) -> q p hl t",
                                         p=4, hl=2)

            SO = 256.0

            def emit_oTf8_requant(p):
                src_v = oTf[0][:].rearrange(
                    "q (p t) -> q p t", t=ROWS)[:, p, :]
                nc.vector.tensor_scalar_mul(
                    oTf8v()[:, p, 0, :], src_v, SO)
                nc.vector.scalar_tensor_tensor(
                    oTf8v()[:, p, 1, :], src_v, SO,
                    oTf8v()[:, p, 0, :],
                    mybir.AluOpType.mult, mybir.AluOpType.subtract)

            def pass0_thunks(ci):
                sbi, doc = divmod(ci, 2)
                holder = {}
                steps = [(kind, p) for p in (0, 2)
                         for kind in ("p0", "p3", "p4")]

                def mk(si):
                    kind, p = steps[si]

                    def t():
                        if si == 0:
                            holder["pout"] = ps.tile(
                                [128, 512], F32, tag="ps_a", bufs=2,
                                name=f"p0_{ci}")
                        ohl = 1 if kind == "p4" else 0
                        whl = 1 if kind == "p3" else 0
                        lhs = oTf8v()[:, p:p + 2, ohl,
                                      sbi * 128:(sbi + 1) * 128]
                        rhs = wo8v()[:, p:p + 2, whl,
                                     doc * 512:(doc + 1) * 512]
                        nc.tensor.matmul(
                            holder["pout"][:], lhs, rhs, perf_mode=DR,
                            start=(si == 0), stop=(si == len(steps) - 1))
                        if si == len(steps) - 1:
                            # the Act engine is idle after the last exp;
                            # copying there keeps the DVE free for the
                            # final adds
                            nc.scalar.mul(o0sb[ci][:], holder["pout"][:],
                                          1.0 / (SO * SW))
                    return t
                return [(107, f"p0_{ci}", mk(si))
                        for si in range(len(steps))]

            def emit_pass1_half(r):
                for sbi in (2 * r, 2 * r + 1):
                    outt = sb.tile([128, DO], BF16, tag="osb", bufs=4,
                                   name=f"outt{sbi}")
                    for doc in range(2):
                        # alternate psum pools: the attention po slots are
                        # free by now, so pass 1 gets an effective 4-deep
                        # rotation
                        tag = "ps_a" if (2 * sbi + doc) % 2 == 0 else "ps_o"
                        pout = ps.tile([128, 512], F32, tag=tag, bufs=2,
                                       name=f"p1_{sbi}_{doc}")
                        for p in range(4):
                            nc.tensor.matmul(
                                pout[:],
                                oTf[(1, r)][
                                    :, p * (ROWS // 2)
                                    + (sbi - 2 * r) * 128:
                                    p * (ROWS // 2)
                                    + (sbi - 2 * r + 1) * 128],
                                wo_p[1][p][:, doc * 512:(doc + 1) * 512],
                                start=(p == 0), stop=(p == 3))
                        nc.vector.tensor_add(
                            outt[:, doc * 512:(doc + 1) * 512], pout[:],
                            o0sb[sbi * 2 + doc][:])
                        nc.sync.dma_start(
                            out_d[sbi * 128:(sbi + 1) * 128,
                                  doc * 512:(doc + 1) * 512],
                            outt[:, doc * 512:(doc + 1) * 512])

            # ---- schedule ----
            # Warm the PE clock while the first DMAs land.
            for wi in range(16):
                wps = ps.tile([64, 64], F32, tag="ps_s", bufs=2,
                              name=f"swarm{wi}")
                nc.tensor.matmul(wps[:], ones_b[0:1, :], ones_b[0:1, :],
                                 start=True, stop=True)
            # Only chunk 0's q/k/v are emitted eagerly; everything else
            # rations through the filler queue with just-in-time flush
            # hooks at the score blocks that need them.
            # chunk 0 eagerly, with q0/k0 interleaved per d-quarter so both
            # trail the arriving x quarters instead of serializing
            q0_th = qk_part_thunks(0, "q")
            k0_th = qk_part_thunks(0, "k")
            for j in range(4):
                for th in (q0_th, k0_th):
                    for s in range(3):
                        th[3 * j + s][2]()
            for e in v_part_thunks(0):
                e[2]()
            # queue order tracks the deadline order: batch 0's k/v/q
            # chunks are consumed across units 0-3, batch 1's k/v land
            # together at unit 8 and its q chunks one unit apiece after.
            for tci in range(1, 4):
                fill_q.extend(qk_part_thunks(tci, "k"))
                fill_q.extend(v_part_thunks(tci))
                fill_q.extend(qk_part_thunks(tci, "q"))
            for tci in range(4, 8):
                fill_q.extend(qk_part_thunks(tci, "k"))
                fill_q.extend(v_part_thunks(tci))
            for tci in range(4, 8):
                fill_q.extend(qk_part_thunks(tci, "q"))

            def mk_flush(labels):
                def h():
                    for lb in labels:
                        flush_through(lb)
                return h

            # JIT flush hooks. Score block i of unit (b,h,qc) reads k
            # chunk b*4 + i//2, and the scores for blocks 0/1 are emitted
            # two blocks into unit u-1 (pre_s fires there). The AV for
            # block i reads v chunk b*4 + i//2 at the block itself
            # (at_hook fires just before the AV). The unit's own q chunk
            # must land before its first scores.
            for u in range(len(sched)):
                b, h, qc = sched[u]
                qchunk = b * 4 + qc
                pre_labels = {}
                for i in (0, 2, 4, 6):
                    kc = b * 4 + i // 2
                    if kc > 0:
                        pre_labels.setdefault(i, []).append(f"k{kc}")
                if qchunk > 0:
                    pre_labels.setdefault(0, []).append(f"q{qchunk}")
                for i, labels in pre_labels.items():
                    pre_s_hook[(u, i)] = mk_flush(labels)
                for i in (0, 2, 4, 6):
                    vc = b * 4 + i // 2
                    if vc > 0:
                        at_hook.setdefault((u, i), []).append(
                            mk_flush([f"v{vc}"]))

            def h0_done():
                # unit 11's obc has just been shipped by emit_tail_b at
                # (12, i==2)
                emit_a2a0()
                emit_oTf0()
            at_hook.setdefault((12, 3), []).append(h0_done)
            for pi, (uu, ii) in enumerate(((13, 0), (13, 4), (14, 0),
                                           (14, 4))):
                at_hook.setdefault((uu, ii), []).append(
                    (lambda p: lambda: emit_oTf8_requant(p))(pi))

            run_units()
            # unit 15's tail, fully pipelined per row-half: each half's
            # reciprocal/normalize/transpose/copy/ship chain runs before
            # the next half's normalize, so the first h1 row-half
            # collective fires as early as possible. Then the even-heads
            # projection pass bridges the collective latency (keeping the
            # PE clock at 2.4 GHz for pass 1).
            fb, fh, fqc, fobsb = pending_tail_b[0]
            pending_tail_b[0] = None
            fshard = fb * (S // TCH) + fqc
            fobc = sb.tile([64, TCH], BF16, tag="obc", bufs=3,
                           name="obc_final")
            for qb0, qb1 in ((0, 2), (2, 4)):
                pt = ps.tile([64, 2 * 128], BF16, tag="ps_a",
                             bufs=2, name=f"ptf{qb0}")
                for qb in range(qb0, qb1):
                    nc.tensor.matmul(
                        pt[:, (qb - qb0) * 128:(qb - qb0 + 1) * 128],
                        fobsb[:, qb * 64:(qb + 1) * 64], ident[:],
                        is_transpose=True, start=True, stop=True)
                c0, c1 = qb0 * 128, qb1 * 128
                nc.vector.tensor_copy(fobc[:, c0:c1], pt[:])
                nc.sync.dma_start(bnc_in1[qb0 // 2][fshard, :, :],
                                  fobc[:, c0:c1])
            emit_a2a1(0)
            emit_a2a1(1)
            emit_oTf1(0)
            emit_oTf1(1)
            flush_all()
            # the whole even-heads projection runs here: it bridges the
            # collective latency (PE clock stays hot) and pass 1's data
            # is not ready any earlier anyway
            for ci in range(8):
                for e in pass0_thunks(ci):
                    e[2]()
            emit_pass1_half(0)
            emit_pass1_half(1)

    nc.compile()
    return nc


def _get_nc():
    if "nc" not in _cache:
        _cache["nc"] = _build()
    return _cache["nc"]


def _f8(a):
    return a.astype(ml_dtypes.float8_e4m3)


def _hilo(a, scale):
    hi = _f8(a * scale)
    lo = _f8(a * scale - hi.astype(np.float32))
    return hi, lo


def _dshuffle(w):
    # [D, 128] -> [128, 8, 128] with out[p, dc, e] = w[dc*128 + p, e]
    return w.reshape(8, 128, 128).transpose(1, 0, 2)


def _in_maps(x, Wq, Wk, Wv, Wo):
    bf16 = ml_dtypes.bfloat16
    xT = x.reshape(T, D).T.astype(np.float32)
    xhi, xlo = _hilo(xT, SX)
    # interleave hi/lo per 128-row d-block: row (dc*2+hl)*128+p
    xt = np.ascontiguousarray(
        np.stack([xhi.reshape(8, 128, T), xlo.reshape(8, 128, T)],
                 axis=1).reshape(2 * D, T))
    wo = np.ascontiguousarray(Wo.astype(bf16))
    WoF = Wo.astype(np.float32)
    wo8 = np.zeros((128, 4 * 2 * DO), dtype=ml_dtypes.float8_e4m3)
    w8hi, w8lo = _hilo(WoF, SW)
    for p in range(4):
        rows = np.r_[(4 * p) * 64:(4 * p) * 64 + 64,
                     (4 * p + 2) * 64:(4 * p + 2) * 64 + 64]
        wo8[:, p * 2048:p * 2048 + 1024] = w8hi[rows]
        wo8[:, p * 2048 + 1024:p * 2048 + 2048] = w8lo[rows]
    ident = np.eye(128, dtype=bf16)
    maps = []
    for c in range(NCORES):
        h0, h1 = HPC * c, HPC * c + 1
        blocks = []
        for W in (Wq, Wk, Wv):
            W2 = np.concatenate([W[h0], W[h1]], axis=1).astype(np.float32)
            whi, wlo = _hilo(W2, SW)
            hi_s = _dshuffle(whi)   # [128, 8, 128]
            lo_s = _dshuffle(wlo)
            # per d-block: [W_hi | dW]
            blocks.append(np.stack([hi_s, lo_s], axis=2))
        wqkv = np.concatenate(blocks, axis=1)  # [128, 24, 2, 128]
        wqkv = wqkv.reshape(128, 3 * 8 * 2 * 128)
        maps.append({
            "xt": xt,
            "wqkv": np.ascontiguousarray(wqkv),
            "wo": wo,
            "wo8": wo8,
            "ident": ident,
        })
    return maps


def kernel(x, Wq, Wk, Wv, Wo, **_):
    nc = _get_nc()
    res = bass_utils.run_bass_kernel_spmd(
        nc, _in_maps(x, Wq, Wk, Wv, Wo), core_ids=list(range(NCORES)))
    out = np.concatenate(
        [res.results[c]["out"].astype(np.float32) for c in range(NCORES)],
        axis=0)
    return out.reshape(B, S, DO)
